# revision 19
# baseline (speedup 1.0000x reference)
"""Two-layer GCN (nn_Method_GCN_11098195493080) as a Bass/Tile kernel on 8
Trainium2 NeuronCores.

Strategy (follows the 1D graph-partition sharding hint):
  - Nodes sharded 8 ways; edges owned by the destination core.
  - Layer 1: y = dinv * (x_shard @ W1) on PE (bf16), AllGather -> full
    message table in every core's HBM (the halo exchange). Layer 2
    applies W2 *before* the halo exchange (propagation commutes with the
    weight multiply), so its table rows are only fout wide.
  - Tables are packed: rows hold only the real payload (64 resp. 32
    bf16), and the 8 cores' rows are interleaved at 4-row granularity
    (tpos = (c//4)*4*nsh + q*4 + c%4) so that every 256B gather block
    holds 2 (layer 1) resp. 4 (layer 2) node rows whose sub-row lane is
    a function of the *source core* only. That keeps the per-(tile,
    lane-group) load balanceable host-side and int16 gather indices in
    range with at most 2 address chunks.
  - Aggregation (both layers): batched dma_gather fetches 256B blocks
    edge-major; a one-hot ST[128 edges, 128 dst] built on the vector
    engine turns the segment sum into PE matmuls accumulated in PSUM,
    with the lane selecting the rhs sub-row. Self loops are added with
    one extra identity matmul per tile. Gathers and the ST build are
    issued one supertile ahead (software pipelining) so the SDMA drain,
    the DVE is_equal and the PE matmuls overlap.
  - Host-side work is integer graph partitioning (shard, bucket, balance,
    bincount for degrees); all float math runs on the NeuronCores.
"""

import numpy as np
import ml_dtypes

import concourse.bass as bass
import concourse.bacc as bacc
import concourse.mybir as mybir
import concourse.tile as tile
from concourse import bass_utils
from concourse.masks import make_identity

F32 = mybir.dt.float32
BF16 = mybir.dt.bfloat16
I16 = mybir.dt.int16
AF = mybir.ActivationFunctionType
OP = mybir.AluOpType
NPBF16 = ml_dtypes.bfloat16

N_CORES = 8
NG = 4                # gather groups per tile (lane/chunk classes)
P = 128               # partitions / dst-tile size
PAD_DST = 254.0       # dst_local value whose one-hot row is all-zero
SLOT_CAP = 12288      # max gathered edge slots per supertile buffer
TROW = 128            # gathered block width in bf16 elems (= 256B)


def _balance_core(d8, nt, cap_nodes=P):
    """Assign nodes (rows of d8 = per-source-core in-degree) to nt tiles,
    balancing per-(tile, source-core) edge loads. Returns tile_of."""
    n = d8.shape[0]
    order = np.argsort(-d8.sum(axis=1), kind="stable")
    loads = np.zeros((nt, d8.shape[1]), dtype=np.int64)
    counts = np.zeros(nt, dtype=np.int64)
    tile_of = np.empty(n, dtype=np.int64)
    full_penalty = np.zeros(nt, dtype=np.int64)
    for node in order:
        d = d8[node]
        score = (loads + d).max(axis=1) + full_penalty
        t = int(np.argmin(score))
        loads[t] += d
        tile_of[node] = t
        counts[t] += 1
        if counts[t] >= cap_nodes:
            full_penalty[t] = 1 << 40
    return tile_of


class Layer:
    """Per-layer gather/scatter schedule (supertile-major block layout)."""

    def __init__(self, plan, g_edge, idx_edge, lane_of_g, chunk_of_g,
                 n_addr_rows, width):
        nt, n_cores = plan.nt, plan.n_cores
        self.lane_of_g = lane_of_g
        self.chunk_of_g = chunk_of_g
        self.n_addr_rows = n_addr_rows   # rows per address chunk in table
        self.width = width

        key = (plan.d_owner * nt + plan.tile_id) * NG + g_edge
        # secondary sort on the gather index: ascending addresses within a
        # segment improve HBM row locality during the SDMA drain
        order = np.lexsort((idx_edge, key))
        sorted_idx = idx_edge[order]
        sorted_dloc = plan.dloc[order]
        ngroups = n_cores * nt * NG
        sizes = np.bincount(key, minlength=ngroups).reshape(n_cores, nt, NG)
        gstart = np.zeros(ngroups + 1, dtype=np.int64)
        np.cumsum(sizes.reshape(-1), out=gstart[1:])

        nb = (sizes.max(axis=0) + P - 1) // P             # [nt, NG]
        nb[:, 0] = np.maximum(nb[:, 0], 1)
        self.nb = nb
        tile_blocks = nb.sum(axis=1)

        # supertiles by cumulative block count
        self.supertiles = []
        t0 = 0
        while t0 < nt:
            t1 = t0 + 1
            tot = int(tile_blocks[t0])
            while t1 < nt and tot + int(tile_blocks[t1]) <= SLOT_CAP // P:
                tot += int(tile_blocks[t1])
                t1 += 1
            self.supertiles.append((t0, t1))
            t0 = t1
        self.max_sg_blocks = max(
            int(tile_blocks[a:b].sum()) for a, b in self.supertiles
        )

        # supertile-major positions: (s | g | t | b)
        self.sg_off = [0]
        self.gspan = []      # per s, per g: (lo, hi) relative to supertile
        self.mmpos = {}      # (t, g) -> block pos rel. to supertile start
        for (a, b) in self.supertiles:
            pos = 0
            spans = []
            for g in range(NG):
                lo = pos
                for t in range(a, b):
                    self.mmpos[(t, g)] = pos
                    pos += int(nb[t, g])
                spans.append((lo, pos))
            self.gspan.append(spans)
            self.sg_off.append(self.sg_off[-1] + pos)
        self.total_blocks = self.sg_off[-1]
        self.total_slots = self.total_blocks * P

        # per-core idx (int16, wrapped+replicated) and dstl (bf16) arrays
        self.idx16 = []
        self.dstl = []
        for c in range(n_cores):
            ia = np.zeros(self.total_slots, dtype=np.int64)
            da = np.full(self.total_blocks * P, PAD_DST, dtype=np.float32)
            for si, (a, b) in enumerate(self.supertiles):
                base = self.sg_off[si]
                for g in range(NG):
                    for t in range(a, b):
                        gi = (c * nt + t) * NG + g
                        lo, hi = gstart[gi], gstart[gi + 1]
                        o = (base + self.mmpos[(t, g)]) * P
                        n = int(hi - lo)
                        ia[o:o + n] = sorted_idx[lo:hi]
                        da[o:o + n] = sorted_dloc[lo:hi]
            assert ia.max(initial=0) < 32768
            self.idx16.append(np.ascontiguousarray(
                np.tile(ia.astype(np.int16).reshape(-1, 16).T, (P // 16, 1))
            ))
            self.dstl.append(np.ascontiguousarray(
                da.reshape(-1, P).T
            ).astype(NPBF16))


class Plan:
    """Static, core-uniform schedule derived from the (integer) graph."""

    def __init__(self, n_nodes, fin, hid, fout, edge_index, n_cores=N_CORES,
                 nt=None):
        assert n_nodes % n_cores == 0
        self.n_nodes = n_nodes
        self.fin, self.hid, self.fout = fin, hid, fout
        self.n_cores = n_cores
        self.base = n_nodes // n_cores
        min_nt = (self.base + P - 1) // P
        self.nt = nt if nt is not None else min_nt + 2 + (min_nt + 63) // 64
        assert self.nt * P >= self.base
        self.nsh = self.nt * P
        self.ntab = self.nsh * n_cores
        assert 2 * self.nsh <= 32767, "chunk must fit int16 gather index"
        assert fin % P == 0
        self.kch = fin // P

        src = np.asarray(edge_index[0], dtype=np.int64)
        dst = np.asarray(edge_index[1], dtype=np.int64)
        s_owner = src // self.base
        self.d_owner = dst // self.base

        # pass 1: per-core node->tile balance on per-source-core in-degree
        tile_of = np.empty(n_nodes, dtype=np.int64)
        for c in range(n_cores):
            sel = self.d_owner == c
            dl = dst[sel] - c * self.base
            sc = s_owner[sel]
            d8 = np.zeros((self.base, n_cores), dtype=np.int64)
            np.add.at(d8, (dl, sc), 1)
            tile_of[c * self.base:(c + 1) * self.base] = _balance_core(
                d8, self.nt
            )

        # pass 2: greedy class (p mod 4) assignment per node, balancing
        # lane counts per destination (core, tile) bucket for both layers.
        # L1 gather classes are (table half, q parity): the node's half is
        # fixed by its tile, so cnt1 tracks parity per (bucket, half).
        nt = self.nt
        self.tb = min(nt // 2, 63)
        self.tbq = self.tb * P
        t_d = tile_of[dst]
        dbkt = self.d_owner * nt + t_d                    # [E]
        outdeg = np.bincount(src, minlength=n_nodes)
        eorder = np.argsort(src, kind="stable")
        estart = np.zeros(n_nodes + 1, dtype=np.int64)
        np.cumsum(outdeg, out=estart[1:])
        dbkt_s = dbkt[eorder]
        cnt1 = np.zeros((n_cores * nt, 2, 2), dtype=np.int64)
        cnt2 = np.zeros((n_cores * nt, 4), dtype=np.int64)
        cap = np.full((n_nodes // self.base * 0 + n_cores, nt, 4), P // 4,
                      dtype=np.int64)
        cls = np.empty(n_nodes, dtype=np.int64)
        BIG = 1 << 40
        for u in np.argsort(-outdeg, kind="stable"):
            c = u // self.base
            t = tile_of[u]
            hu = int(t >= self.tb)
            bs = dbkt_s[estart[u]:estart[u + 1]]
            c1 = cnt1[bs, hu, :].sum(axis=0)
            c2 = cnt2[bs, :].sum(axis=0)
            cost = c1[[0, 1, 0, 1]] + c2
            cost = np.where(cap[c, t, :] > 0, cost, BIG)
            k = int(np.argmin(cost))
            cls[u] = k
            np.add.at(cnt1, (bs, hu, k % 2), 1)
            np.add.at(cnt2, (bs, k), 1)
            cap[c, t, k] -= 1
        # final slots: class k occupies positions p with p%4 == k
        self.slot_of = np.empty((n_cores, self.base), dtype=np.int64)
        for c in range(n_cores):
            for t in range(nt):
                sel = np.where(tile_of[c * self.base:(c + 1) * self.base]
                               == t)[0]
                kk = cls[c * self.base + sel]
                p = np.empty(len(sel), dtype=np.int64)
                for k in range(4):
                    m = kk == k
                    p[m] = k + 4 * np.arange(m.sum())
                self.slot_of[c, sel] = t * P + p

        d_slot = self.slot_of[self.d_owner, dst - self.d_owner * self.base]
        q_src = self.slot_of[s_owner, src - s_owner * self.base]
        self.tile_id = d_slot // P
        self.dloc = (d_slot % P).astype(np.float32)

        # tables are split in two halves at tile boundary tb (q < tbq is
        # half A) so the halo AllGathers can be chunked and overlapped with
        # compute.  Each half is rank-major: row = s_owner*half_len + qh.
        tbq = self.tbq
        h = (q_src >= tbq).astype(np.int64)
        qh = q_src - h * tbq
        half_len = np.where(h == 0, tbq, self.nsh - tbq)
        r = s_owner * half_len + qh
        # layer 1: 256B block = 2 rows of 64 bf16; chunk = table half
        g1 = h * 2 + (q_src % 2)
        idx1 = r // 2
        # layer 2: 256B block = 4 rows of 32 bf16; single table (rank-major
        # over full shards, one un-chunked AllGather), one address chunk
        g2 = q_src % 4
        idx2 = (s_owner * self.nsh + q_src) // 4

        self.L1 = Layer(self, g1, idx1,
                        lane_of_g=[0, 1, 0, 1], chunk_of_g=[0, 0, 1, 1],
                        n_addr_rows=0, width=hid)
        self.L2 = Layer(self, g2, idx2,
                        lane_of_g=[0, 1, 2, 3], chunk_of_g=[0, 0, 0, 0],
                        n_addr_rows=0, width=fout)

        # degrees (with self loop), per core wrapped [128, nt], slot order
        deg = np.bincount(dst, minlength=n_nodes).astype(np.float32) + 1.0
        self.degw = np.ones((n_cores, P, self.nt), dtype=np.float32)
        for c in range(n_cores):
            d = np.ones(self.nsh, dtype=np.float32)
            d[self.slot_of[c]] = deg[c * self.base:(c + 1) * self.base]
            self.degw[c] = d.reshape(self.nt, P).T


def build_nc(plan: Plan, b1_zero=True, b2_zero=True):
    nc = bacc.Bacc(
        "TRN2",
        target_bir_lowering=False,
        debug=False,
        enable_asserts=False,
        num_devices=plan.n_cores,
        num_swdge_queues=NG,
        dynamic_dma_scratch_size=16384,
    )
    fin, hid, fout = plan.fin, plan.hid, plan.fout
    nt, nsh, kch = plan.nt, plan.nsh, plan.kch

    xT = nc.dram_tensor("xT", [fin, nsh], BF16, kind="ExternalInput")
    degw = nc.dram_tensor("degw", [P, nt], F32, kind="ExternalInput")
    w1 = nc.dram_tensor("w1", [fin, hid], BF16, kind="ExternalInput")
    b1 = nc.dram_tensor("b1", [1, hid], F32, kind="ExternalInput")
    w2 = nc.dram_tensor("w2", [hid, fout], F32, kind="ExternalInput")
    b2 = nc.dram_tensor("b2", [1, fout], F32, kind="ExternalInput")
    idx_d = {}
    dstl_d = {}
    for nm, L in (("1", plan.L1), ("2", plan.L2)):
        idx_d[nm] = nc.dram_tensor(f"idx{nm}", [P, L.total_slots // 16], I16,
                                   kind="ExternalInput")
        dstl_d[nm] = nc.dram_tensor(f"dstl{nm}", [P, L.total_blocks], BF16,
                                    kind="ExternalInput")
    out = nc.dram_tensor("out", [nsh, fout], F32, kind="ExternalOutput")

    rg = [list(range(plan.n_cores))]

    with tile.TileContext(nc) as tc:
        with (
            tc.tile_pool(name="const", bufs=1) as cp,
            tc.tile_pool(name="dram", bufs=1, space="DRAM") as dp,
        ):
            # ---- constants -------------------------------------------------
            iota = cp.tile([P, P], BF16)
            nc.gpsimd.iota(iota[:], pattern=[[1, P]], base=0,
                           channel_multiplier=0,
                           allow_small_or_imprecise_dtypes=True)
            ident = cp.tile([P, P], F32)
            make_identity(nc, ident[:])
            identb = cp.tile([P, P], BF16)
            nc.vector.tensor_copy(identb[:], ident[:])

            w1sb = cp.tile([P, kch, hid], BF16)
            nc.sync.dma_start(
                w1sb[:], w1.ap().rearrange("(a p) f -> p a f", p=P)
            )
            w2sb = cp.tile([hid, fout], F32)
            nc.sync.dma_start(w2sb[:], w2.ap())
            b1row = cp.tile([P, hid], F32)
            nc.sync.dma_start(b1row[:], b1.ap().to_broadcast([P, hid]))
            b2row = cp.tile([P, fout], F32)
            nc.sync.dma_start(b2row[:], b2.ap().to_broadcast([P, fout]))

            degt = cp.tile([P, nt], F32)
            nc.sync.dma_start(degt[:], degw.ap())
            rec = cp.tile([P, nt], F32)
            nc.vector.reciprocal(rec[:], degt[:])
            dinv = cp.tile([P, nt], F32)
            nc.scalar.activation(dinv[:], rec[:], AF.Sqrt)

            ssum_all = cp.tile([P, nt], F32)
            y_loc = cp.tile([P, nt, hid], BF16)
            zq_loc = cp.tile([P, nt, fout], BF16)
            out_loc = cp.tile([P, nt, fout], F32)

            tbq = plan.tbq
            y_bounce_a = dp.tile([tbq, hid], BF16)
            y_bounce_b = dp.tile([nsh - tbq, hid], BF16)
            z_bounce = dp.tile([nsh, fout], BF16)
            # half-tables: rank-major rows, 256B blocks of 2 (L1) rows
            table1a = nc.dram_tensor("table1a", [8 * tbq // 2, TROW], BF16,
                                     kind="Internal", addr_space="Shared")
            table1b = nc.dram_tensor("table1b", [8 * (nsh - tbq) // 2, TROW],
                                     BF16, kind="Internal",
                                     addr_space="Shared")
            # single L2 table, A blocks first then B blocks
            table2 = nc.dram_tensor("table2", [2 * nsh, TROW], BF16,
                                    kind="Internal", addr_space="Shared")

            # ---- phase 1: y = dinv * (x @ W1) ------------------------------
            tb = plan.tb

            def store_halves(buf_a, buf_b, loc, t0, t1):
                """DMA loc[:, t0:t1] into the tile-split bounce halves."""
                if t0 < tb:
                    hi = min(t1, tb)
                    nc.sync.dma_start(
                        buf_a[:].rearrange("(t p) f -> p t f", p=P)
                        [:, t0:hi, :],
                        loc[:, t0:hi, :],
                    )
                if t1 > tb:
                    lo = max(t0, tb)
                    nc.sync.dma_start(
                        buf_b[:].rearrange("(t p) f -> p t f", p=P)
                        [:, lo - tb:t1 - tb, :],
                        loc[:, lo:t1, :],
                    )

            WB = 8
            with (
                tc.tile_pool(name="xload", bufs=3) as xp,
                tc.tile_pool(name="ps1", bufs=8, space="PSUM") as pp1,
            ):
                xTap = xT.ap().rearrange("(a p) n -> p a n", p=P)
                ag1a_fired = False
                for wb in range(0, nt, WB):
                    nwin = min(WB, nt - wb)
                    xt = xp.tile([P, kch, P * WB], BF16, tag="xt")
                    nc.sync.dma_start(
                        xt[:, :, : P * nwin],
                        xTap[:, :, wb * P:(wb + nwin) * P],
                    )
                    for w in range(nwin):
                        t = wb + w
                        ps = pp1.tile([P, hid], F32, tag="ps1")
                        for a in range(kch):
                            nc.tensor.matmul(
                                ps[:],
                                lhsT=xt[:, a, w * P:(w + 1) * P],
                                rhs=w1sb[:, a, :],
                                start=(a == 0),
                                stop=(a == kch - 1),
                            )
                        nc.vector.tensor_scalar(
                            out=y_loc[:, t, :], in0=ps[:],
                            scalar1=dinv[:, t:t + 1], scalar2=None,
                            op0=OP.mult,
                        )
                    store_halves(y_bounce_a, y_bounce_b, y_loc, wb, wb + nwin)
                    if not ag1a_fired and wb + nwin >= tb:
                        ag1a_fired = True
                        nc.gpsimd.collective_compute(
                            "AllGather", OP.bypass, replica_groups=rg,
                            ins=[y_bounce_a.opt()], outs=[table1a.ap()],
                        )
            nc.gpsimd.collective_compute(
                "AllGather", OP.bypass, replica_groups=rg,
                ins=[y_bounce_b.opt()], outs=[table1b.ap()],
            )

            # ---- aggregation pass (software-pipelined supertiles) ----------
            def aggregate(L, table_aps, idx_dram, dstl_dram, self_rhs,
                          epilogue, post=None):
                W = L.width
                with (
                    tc.tile_pool(name="lcon", bufs=1) as lc,
                    tc.tile_pool(name="gath", bufs=3) as gp,
                    tc.tile_pool(name="ixp", bufs=3) as ixp,
                    tc.tile_pool(name="stp", bufs=2) as stp,
                    tc.tile_pool(name="ps2", bufs=4, space="PSUM") as pp2,
                    tc.tile_pool(name="eps", bufs=3) as ep,
                    tc.tile_pool(name="psT", bufs=2, space="PSUM") as ppT,
                    tc.tile_pool(name="pso", bufs=2, space="PSUM") as ppo,
                ):
                    dstlsb = lc.tile([P, L.total_blocks], BF16, tag="dstl")
                    nc.sync.dma_start(dstlsb[:], dstl_dram.ap())

                    nS = len(L.supertiles)

                    def issue(si):
                        sgb = L.sg_off[si + 1] - L.sg_off[si]
                        yb = gp.tile([P, L.max_sg_blocks, TROW], BF16,
                                     tag="yb")
                        # stream this supertile's gather indices
                        ixt = ixp.tile([P, L.max_sg_blocks * 8], I16,
                                       tag="ix")
                        nc.sync.dma_start(
                            ixt[:, :sgb * 8],
                            idx_dram.ap()[:, L.sg_off[si] * 8:
                                          L.sg_off[si + 1] * 8],
                        )
                        # one gather instruction per address chunk; split a
                        # single-chunk supertile in two for queue parallelism
                        spans = []
                        for g in range(NG):
                            lo, hi = L.gspan[si][g]
                            if hi == lo:
                                continue
                            ch = L.chunk_of_g[g]
                            if spans and spans[-1][0] == ch \
                                    and spans[-1][2] == lo:
                                spans[-1] = (ch, spans[-1][1], hi)
                            else:
                                spans.append((ch, lo, hi))
                        if len(spans) == 1:
                            ch, lo, hi = spans[0]
                            mid = L.gspan[si][1][1]
                            if lo < mid < hi:
                                spans = [(ch, lo, mid), (ch, mid, hi)]
                        for j, (ch, lo, hi) in enumerate(spans):
                            nc.gpsimd.dma_gather(
                                yb[:, lo:hi, :],
                                table_aps[ch],
                                ixt[:, lo * 8:hi * 8],
                                (hi - lo) * P,
                                (hi - lo) * P,
                                TROW,
                                single_packet=False,
                                queue_num=(2 * si + j) % NG,
                            )
                        st = stp.tile([P, L.max_sg_blocks, P], BF16,
                                      tag="st")
                        o0 = L.sg_off[si]
                        nc.vector.tensor_tensor(
                            out=st[:, :sgb, :],
                            in0=iota[:].rearrange(
                                "p (a f) -> p a f", a=1
                            ).to_broadcast([P, sgb, P]),
                            in1=dstlsb[:, o0:o0 + sgb].rearrange(
                                "p (b o) -> p b o", o=1
                            ).to_broadcast([P, sgb, P]),
                            op=OP.is_equal,
                        )
                        return yb, st

                    from collections import deque
                    pending = deque([issue(0)])
                    if nS > 1:
                        pending.append(issue(1))
                    for si, (t0, t1) in enumerate(L.supertiles):
                        if si + 2 < nS:
                            pending.append(issue(si + 2))
                        yb, st = pending.popleft()
                        for t in range(t0, t1):
                            ps = pp2.tile([P, W], F32, tag="ps2")
                            done = 0
                            for g in range(NG):
                                lane = L.lane_of_g[g]
                                for b in range(int(L.nb[t, g])):
                                    pos = L.mmpos[(t, g)] + b
                                    nc.tensor.matmul(
                                        ps[:], lhsT=st[:, pos, :],
                                        rhs=yb[:, pos,
                                               lane * W:(lane + 1) * W],
                                        start=(done == 0), stop=False,
                                    )
                                    done += 1
                            # self loop: += identity @ self_rhs[t]
                            nc.tensor.matmul(
                                ps[:], lhsT=identb[:],
                                rhs=self_rhs[:, t, :],
                                start=False, stop=True,
                            )
                            epilogue(t, ps, ep, ppT, ppo)
                        if post is not None:
                            post(t0, t1)

            # ---- layer-1 epilogue: zq = (dinv*relu(dinv*s + b1)) @ W2 ------
            # With b1 == 0 the whole epilogue runs on the ACT engine so the
            # vector engine only ever builds one-hot STs (its in-order queue
            # would otherwise serialize PSUM release behind the next ST).
            def epi1(t, ps, ep, ppT, ppo):
                hh = ep.tile([P, hid], F32, tag="hh")
                if b1_zero:
                    nc.scalar.activation(hh[:], ps[:], AF.Relu,
                                         scale=dinv[:, t:t + 1])
                else:
                    a2 = ep.tile([P, hid], F32, tag="a2")
                    nc.vector.scalar_tensor_tensor(
                        out=a2[:], in0=ps[:], scalar=dinv[:, t:t + 1],
                        in1=b1row[:], op0=OP.mult, op1=OP.add,
                    )
                    nc.scalar.activation(hh[:], a2[:], AF.Relu)
                pT = ppT.tile([hid, P], F32, tag="pT")
                nc.tensor.transpose(out=pT[:], in_=hh[:], identity=ident[:])
                hhT = ep.tile([hid, P], F32, tag="hhT")
                nc.scalar.copy(hhT[:], pT[:])
                po = ppo.tile([P, fout], F32, tag="po")
                nc.tensor.matmul(po[:], lhsT=hhT[:], rhs=w2sb[:],
                                 start=True, stop=True)
                nc.scalar.mul(zq_loc[:, t, :], po[:], dinv[:, t:t + 1])

            def post1(t0, t1):
                nc.sync.dma_start(
                    z_bounce[:].rearrange("(t p) f -> p t f", p=P)
                    [:, t0:t1, :],
                    zq_loc[:, t0:t1, :],
                )

            aggregate(plan.L1, [table1a.ap(), table1b.ap()], idx_d["1"],
                      dstl_d["1"], y_loc, epi1, post=post1)
            nc.gpsimd.collective_compute(
                "AllGather", OP.bypass, replica_groups=rg,
                ins=[z_bounce.opt()], outs=[table2.ap()],
            )

            # ---- layer-2 epilogue: log_softmax(dinv*s + b2) ----------------
            # o values are O(10), so exp/sum in f32 is safe without the
            # usual max subtraction; the log-sum-exp is applied at the end.
            def epi2(t, ps, ep, ppT, ppo):
                if b2_zero:
                    nc.scalar.mul(out_loc[:, t, :], ps[:], dinv[:, t:t + 1])
                else:
                    nc.vector.scalar_tensor_tensor(
                        out=out_loc[:, t, :], in0=ps[:],
                        scalar=dinv[:, t:t + 1],
                        in1=b2row[:], op0=OP.mult, op1=OP.add,
                    )
                e = ep.tile([P, fout], F32, tag="e")
                nc.scalar.activation(e[:], out_loc[:, t, :], AF.Exp,
                                     accum_out=ssum_all[:, t:t + 1])

            # deferred log-sum-exp per supertile: out -= log(ssum)
            lse_all = cp.tile([P, nt], F32)

            def post2(t0, t1):
                nc.scalar.activation(lse_all[:, t0:t1], ssum_all[:, t0:t1],
                                     AF.Ln)
                nc.vector.tensor_tensor(
                    out=out_loc[:, t0:t1, :], in0=out_loc[:, t0:t1, :],
                    in1=lse_all[:, t0:t1].rearrange("p (t o) -> p t o", o=1)
                    .to_broadcast([P, t1 - t0, fout]),
                    op=OP.subtract,
                )
                nc.sync.dma_start(
                    out.ap().rearrange("(t p) f -> p t f", p=P)
                    [:, t0:t1, :],
                    out_loc[:, t0:t1, :],
                )

            aggregate(plan.L2, [table2.ap()], idx_d["2"], dstl_d["2"],
                      zq_loc, epi2, post=post2)

    nc.compile()
    return nc


def make_in_maps(plan: Plan, x, W1, b1, W2, b2):
    x = np.asarray(x, dtype=np.float32)
    w1b = np.ascontiguousarray(W1, dtype=np.float32).astype(NPBF16)
    in_maps = []
    for c in range(plan.n_cores):
        xT = np.zeros((plan.fin, plan.nsh), dtype=NPBF16)
        xs = x[c * plan.base:(c + 1) * plan.base, :].astype(NPBF16)
        xT[:, plan.slot_of[c]] = xs.T
        m = {
            "xT": xT,
            "degw": plan.degw[c],
            "w1": w1b,
            "b1": np.asarray(b1, dtype=np.float32).reshape(1, -1),
            "w2": np.ascontiguousarray(W2, dtype=np.float32),
            "b2": np.asarray(b2, dtype=np.float32).reshape(1, -1),
            "idx1": plan.L1.idx16[c],
            "dstl1": plan.L1.dstl[c],
            "idx2": plan.L2.idx16[c],
            "dstl2": plan.L2.dstl[c],
        }
        in_maps.append(m)
    return in_maps


_CACHE = {}


def _get_compiled(n_nodes, fin, hid, fout, edge_key, edge_index,
                  b1_zero, b2_zero):
    key = (n_nodes, fin, hid, fout, edge_key, b1_zero, b2_zero)
    if key not in _CACHE:
        plan = Plan(n_nodes, fin, hid, fout, edge_index)
        nc = build_nc(plan, b1_zero, b2_zero)
        _CACHE[key] = (plan, nc)
    return _CACHE[key]


def kernel(x, edge_index, W1, b1, W2, b2, _trace=False):
    x = np.asarray(x)
    edge_index = np.asarray(edge_index)
    n_nodes, fin = x.shape
    hid = np.asarray(W1).shape[1]
    fout = np.asarray(W2).shape[1]
    edge_key = hash(edge_index.tobytes())
    b1_zero = bool(np.all(np.asarray(b1) == 0))
    b2_zero = bool(np.all(np.asarray(b2) == 0))
    plan, nc = _get_compiled(n_nodes, fin, hid, fout, edge_key, edge_index,
                             b1_zero, b2_zero)
    in_maps = make_in_maps(plan, x, W1, b1, W2, b2)
    res = bass_utils.run_bass_kernel_spmd(
        nc, in_maps, core_ids=list(range(plan.n_cores)), trace=_trace
    )
    parts = [
        res.results[c]["out"][plan.slot_of[c], :]
        for c in range(plan.n_cores)
    ]
    out = np.concatenate(parts, axis=0).astype(np.float32)
    kernel.last_results = res
    return out

